# revision 41
# baseline (speedup 1.0000x reference)
"""Trainium2 Bass kernel for nn_ArflowSparseMoeBlock (8-expert top-2 MoE, 4-layer ELU MLP).

Strategy (8 NeuronCores, expert-parallel with token dispatch):
  - Each core owns ONE expert's weights (w1..b4 sharded on the leading E axis).
  - The router (x @ gate_w, softmax, top-2, renormalize) runs on host exactly
    as the reference does (jax f32 on CPU), because its result IS the sharding
    decision: tokens are dispatched to the core owning each selected expert.
    Each core receives only its expert's ~T*K/E tokens (padded to capacity C),
    pre-transposed to feature-major [D, C] so the whole 4-layer MLP chains
    with zero on-device transposes.
  - The device computes y_e = W4.T elu(W3.T elu(W2.T elu(W1.T x + b1) + b2) + b3)
    for its C-token batch (bf16 matmuls, fp32 accumulation) and returns
    y [O, C] fp32. The host applies bias b4 + routing weights and scatter-adds
    into the full [T, O] output (the "unshard" step, ~0.2 MFLOP).
  - No device collectives at all; w1 (25 MB bf16) streams from HBM in 2 MB
    chunks overlapped with the L1 matmul stream.
"""

import numpy as np

import concourse.bass as bass
import concourse.tile as tile
from concourse import bacc, mybir
from concourse.bass_utils import run_bass_kernel_spmd

# Problem constants (hardcoded per harness rules)
D = 12336        # input features
P = 128
DP = 12416       # D padded to 97 * 128
KD = DP // P     # 97 k-tiles
H = 1024         # intermediate features
O = 96           # output features
OP = 128         # O padded to full partition width
E = 8            # experts == cores
TOP_K = 2
N_CORES = 8
MT = H // P      # 8 m-tiles
KG = 8           # w1 k-tiles per streamed DMA chunk (2 MB)

F32 = mybir.dt.float32
BF16 = mybir.dt.bfloat16


def _kgroups():
    """k-tile groups for the w1/x stream: ramped-up sizes so the first
    matmuls start ~1us in (the PE is compute-paced from the start), then
    KG-sized chunks."""
    sizes = [2, 2, 5]   # 9 ramp k-tiles; 97-9 = 88 = 11 full KG=8 groups, so
                        # the mi-major last group is a full 8 k-tiles and its
                        # interleaved drains overlap the MM stream
    groups = []
    k = 0
    for s in sizes:
        if k >= KD:
            break
        n = min(s, KD - k)
        groups.append((k, n))
        k += n
    while k < KD:
        n = min(KG, KD - k)
        groups.append((k, n))
        k += n
    return groups


def build(C):
    """Build the SPMD Bass program (identical graph on all 8 cores) for a
    token capacity of C (multiple of 32, <= 512)."""
    assert C % 32 == 0 and 0 < C <= 512
    nc = bacc.Bacc("TRN2", target_bir_lowering=False, debug=False,
                   num_devices=N_CORES)

    # ---- I/O (all pre-arranged on host, partition-major) ----
    xt = nc.dram_tensor("xt", [P, KD, C], BF16, kind="ExternalInput").ap()
    w1 = nc.dram_tensor("w1", [P, KD, H], BF16, kind="ExternalInput").ap()
    w2 = nc.dram_tensor("w2", [P, MT, H], BF16, kind="ExternalInput").ap()
    w3 = nc.dram_tensor("w3", [P, MT, H], BF16, kind="ExternalInput").ap()
    w4 = nc.dram_tensor("w4", [P, MT, OP], BF16, kind="ExternalInput").ap()
    b1 = nc.dram_tensor("b1", [P, MT], F32, kind="ExternalInput").ap()
    b2 = nc.dram_tensor("b2", [P, MT], F32, kind="ExternalInput").ap()
    b3 = nc.dram_tensor("b3", [P, MT], F32, kind="ExternalInput").ap()
    out_ext = nc.dram_tensor("out", [OP, C], F32, kind="ExternalOutput").ap()

    with tile.TileContext(nc) as tc:
        with (
            tc.tile_pool(name="const", bufs=1) as const,
            tc.tile_pool(name="wstream", bufs=1) as wstream,
            tc.tile_pool(name="xstream", bufs=1) as xstream,
            tc.tile_pool(name="hbuf", bufs=2) as hbuf,
            tc.tile_pool(name="epil", bufs=3) as epil,
            tc.tile_pool(name="outp", bufs=1) as outp,
            tc.tile_pool(name="psum", bufs=8, space="PSUM") as psum,
        ):
            b1_sb = const.tile([P, MT], F32)
            nc.sync.dma_start(out=b1_sb, in_=b1)

            def elu_drain(dst, ps, bias):
                """dst = elu(ps + bias) = min(exp(x+b) - 1, relu(x+b))."""
                a = epil.tile([P, C], F32, tag="elu_a")
                nc.scalar.activation(a, ps, mybir.ActivationFunctionType.Exp,
                                     bias=bias)
                r = epil.tile([P, C], F32, tag="elu_r")
                nc.vector.tensor_scalar(r, ps, scalar1=bias, scalar2=0.0,
                                        op0=mybir.AluOpType.add,
                                        op1=mybir.AluOpType.max)
                return nc.vector.scalar_tensor_tensor(
                    dst, a, -1.0, r,
                    op0=mybir.AluOpType.add,
                    op1=mybir.AluOpType.min)

            # ---------- L1: h1 = elu(w1.T @ x + b1), feature-major ----------
            # k-major streaming for all groups but the last; the last group
            # runs mi-major so each accumulator finishes (and drains) while
            # later mi blocks are still on the PE -- no PE gap into L2.
            h1 = hbuf.tile([P, MT, C], BF16, tag="h", name="h_l1")
            ps = [psum.tile([P, C], F32, tag="sp", name=f"acc1_{mi}")
                  for mi in range(MT)]
            groups = _kgroups()
            tiles = []
            for gi, (k0, kn) in enumerate(groups):
                w1g = wstream.tile([P, KG, H], BF16, tag=f"w1g{gi % 4}",
                                   name=f"w1g_{gi}")
                nc.sync.dma_start(out=w1g[:, :kn, :], in_=w1[:, k0:k0 + kn, :])
                xg = xstream.tile([P, KG, C], BF16, tag=f"xg{gi % 4}",
                                    name=f"xg_{gi}")
                nc.sync.dma_start(out=xg[:, :kn, :], in_=xt[:, k0:k0 + kn, :])
                tiles.append((k0, kn, w1g, xg))
                if gi == len(groups) - 1:
                    break
                for k in range(kn):
                    for mi in range(MT):
                        nc.tensor.matmul(
                            ps[mi],
                            w1g[:, k, mi * P:(mi + 1) * P],
                            xg[:, k, :],
                            start=(k0 + k == 0),
                            stop=False)

            # ---------- late constants (DMA overlaps the L1 stream) ----------
            w2_sb = const.tile([P, MT, H], BF16)
            nc.sync.dma_start(out=w2_sb, in_=w2)
            b2_sb = const.tile([P, MT], F32)
            nc.sync.dma_start(out=b2_sb, in_=b2)
            w3_sb = const.tile([P, MT, H], BF16)
            nc.sync.dma_start(out=w3_sb, in_=w3)
            b3_sb = const.tile([P, MT], F32)
            nc.sync.dma_start(out=b3_sb, in_=b3)
            w4_sb = const.tile([P, MT, OP], BF16)
            nc.sync.dma_start(out=w4_sb, in_=w4)

            # last L1 group, mi-major with interleaved drains
            k0, kn, w1g, xg = tiles[-1]
            for mi in range(MT):
                for k in range(kn):
                    nc.tensor.matmul(
                        ps[mi],
                        w1g[:, k, mi * P:(mi + 1) * P],
                        xg[:, k, :],
                        start=False,
                        stop=(k == kn - 1))
                elu_drain(h1[:, mi, :], ps[mi], b1_sb[:, mi:mi + 1])

            # ---------- L2/L3: mi-major, drains overlap the MM stream ------
            def mid_layer(h_in, w_sb, b_sb, lname):
                h_out = hbuf.tile([P, MT, C], BF16, tag="h", name=f"h_{lname}")
                for mi in range(MT):
                    ps = psum.tile([P, C], F32, tag="sp",
                                   name=f"acc_{lname}_{mi}")
                    for k in range(MT):
                        nc.tensor.matmul(
                            ps,
                            w_sb[:, k, mi * P:(mi + 1) * P],
                            h_in[:, k, :],
                            start=(k == 0), stop=(k == MT - 1))
                    elu_drain(h_out[:, mi, :], ps, b_sb[:, mi:mi + 1])
                return h_out

            h2 = mid_layer(h1, w2_sb, b2_sb, "l2")
            h3 = mid_layer(h2, w3_sb, b3_sb, "l3")

            # ---------- L4: y = w4.T @ h3, feature-major [OP, C] ----------
            ps_y = psum.tile([P, C], F32, tag="sp", name="acc_l4")
            for k in range(MT):
                nc.tensor.matmul(ps_y, w4_sb[:, k, :], h3[:, k, :],
                                 start=(k == 0), stop=(k == MT - 1))
            out_sb = outp.tile([P, C], F32)
            nc.vector.tensor_copy(out_sb, ps_y)
            nc.sync.dma_start(out=out_ext, in_=out_sb)

    nc.compile()
    return nc


_NC_CACHE = {}


def get_nc(C):
    if C not in _NC_CACHE:
        _NC_CACHE[C] = build(C)
    return _NC_CACHE[C]


def route_host(x, gate_w):
    """Replicate the reference router bit-for-bit (jax f32 on CPU):
    returns sel [T, K] int32, top_w [T, K] f32 (renormalized)."""
    try:
        import jax
        import jax.numpy as jnp
        cpu = jax.devices("cpu")[0]
        with jax.default_device(cpu):
            logits = jnp.asarray(x, jnp.float32) @ jnp.asarray(gate_w,
                                                               jnp.float32)
            probs = jax.nn.softmax(logits.astype(jnp.float32), axis=-1)
            top_w, sel = jax.lax.top_k(probs, TOP_K)
            top_w = top_w / jnp.sum(top_w, axis=-1, keepdims=True)
        return np.asarray(sel), np.asarray(top_w, dtype=np.float32)
    except Exception:
        logits = x.astype(np.float64) @ gate_w.astype(np.float64)
        logits -= logits.max(axis=-1, keepdims=True)
        p = np.exp(logits)
        p /= p.sum(axis=-1, keepdims=True)
        sel = np.argsort(-p, axis=-1, kind="stable")[:, :TOP_K]
        tw = np.take_along_axis(p, sel, axis=1)
        tw = (tw / tw.sum(axis=-1, keepdims=True)).astype(np.float32)
        return sel.astype(np.int32), tw


def _pad_rows(a, rows):
    out = np.zeros((rows,) + a.shape[1:], dtype=a.dtype)
    out[:a.shape[0]] = a
    return out


def _pkm(a, dt):
    """[K*P, M] row-major -> [P, K, M] partition-major, cast to dt."""
    kp, m = a.shape
    return np.ascontiguousarray(
        a.reshape(kp // P, P, m).transpose(1, 0, 2)).astype(dt)


def dispatch(hidden_states, gate_w):
    """Host-side routing + per-expert token lists."""
    x = np.asarray(hidden_states, np.float32).reshape(-1, D)
    sel, tw = route_host(x, np.asarray(gate_w, np.float32))
    idxs, cws = [], []
    for e in range(E):
        tok, slot = np.nonzero(sel == e)
        idxs.append(tok)
        cws.append(tw[tok, slot])
    cmax = max(len(i) for i in idxs)
    C = min(512, max(64, -(-cmax // 32) * 32))
    return x, idxs, cws, C


def make_in_maps(x, idxs, w1, b1, w2, b2, w3, b3, w4, C):
    import ml_dtypes
    bf = ml_dtypes.bfloat16
    T = x.shape[0]
    xT = np.zeros((DP, T), np.float32)
    xT[:D] = x.T
    in_maps = []
    for e in range(E):
        xg = np.zeros((DP, C), np.float32)
        n = min(len(idxs[e]), C)
        xg[:, :n] = xT[:, idxs[e][:n]]
        xt_r = np.ascontiguousarray(
            xg.reshape(KD, P, C).transpose(1, 0, 2)).astype(bf)
        w4p = np.zeros((H, OP), np.float32)
        w4p[:, :O] = np.asarray(w4[e], np.float32)
        in_maps.append({
            "xt": xt_r,
            "w1": _pkm(_pad_rows(np.asarray(w1[e], np.float32), DP), bf),
            "w2": _pkm(np.asarray(w2[e], np.float32), bf),
            "w3": _pkm(np.asarray(w3[e], np.float32), bf),
            "w4": _pkm(w4p, bf),
            "b1": np.ascontiguousarray(
                np.asarray(b1[e], np.float32).reshape(MT, P).T),
            "b2": np.ascontiguousarray(
                np.asarray(b2[e], np.float32).reshape(MT, P).T),
            "b3": np.ascontiguousarray(
                np.asarray(b3[e], np.float32).reshape(MT, P).T),
        })
    return in_maps


def combine(results, idxs, cws, b4, T):
    out = np.zeros((T, O), np.float32)
    for e in range(E):
        n = len(idxs[e])
        if n == 0:
            continue
        y = np.asarray(results[e]["out"], np.float32)[:O, :n].T
        out[idxs[e]] += cws[e][:, None] * (y + np.asarray(b4[e], np.float32))
    return out


def _spot_ok(res, in_maps, ntok=3):
    """Cheap integrity check: recompute a few tokens per expert on host from
    the exact bf16 arrays the device consumed; catches transient device
    corruption (expected mismatch is only bf16 rounding, ~1e-2 absmax)."""
    def unpkm(a):
        a = np.asarray(a, np.float32)
        return a.transpose(1, 0, 2).reshape(-1, a.shape[2])

    def unb(b):
        return np.asarray(b, np.float32).T.reshape(-1)

    for e in range(E):
        m = in_maps[e]
        x = unpkm(m["xt"])[:, :ntok]
        h = x
        for wk, bk in (("w1", "b1"), ("w2", "b2"), ("w3", "b3")):
            v = unpkm(m[wk]).T @ h + unb(m[bk])[:, None]
            h = np.where(v > 0, v, np.exp(np.minimum(v, 0)) - 1)
        y_ref = unpkm(m["w4"]).T @ h                      # [OP, ntok]
        y_dev = np.asarray(res.results[e]["out"], np.float32)[:, :ntok]
        err = np.linalg.norm(y_dev - y_ref) / max(np.linalg.norm(y_ref), 1e-6)
        if not np.isfinite(err) or err > 0.05:
            return False
    return True


def _run(hidden_states, gate_w, w1, b1, w2, b2, w3, b3, w4, b4,
         trace=False, tmpdir=None):
    x, idxs, cws, C = dispatch(hidden_states, gate_w)
    nc = get_nc(C)
    in_maps = make_in_maps(x, idxs, w1, b1, w2, b2, w3, b3, w4, C)
    for attempt in range(3):
        res = run_bass_kernel_spmd(nc, in_maps, core_ids=list(range(N_CORES)),
                                   trace=trace, tmpdir=tmpdir)
        if _spot_ok(res, in_maps):
            break
    out = combine(res.results, idxs, cws, b4, x.shape[0])
    bsz = np.asarray(hidden_states).shape[0]
    return out.reshape(bsz, -1, O), res


def kernel(hidden_states, gate_w, w1, b1, w2, b2, w3, b3, w4, b4):
    out, _ = _run(hidden_states, gate_w, w1, b1, w2, b2, w3, b3, w4, b4)
    return out
